# revision 4
# baseline (speedup 1.0000x reference)
"""KAN layer (Chebyshev deg-8) Trainium2 kernel, 8-core data-parallel.

Math: out[b] = sum_n hw[n] * (X @ C.T)[b,n] = X[b,:] @ (C.T @ hw)
            = sum_d sum_k W[d,k] * T_k(tanh(x[b,d])),  W[d,k]=(C.T@hw)[d*9+k]

Device evaluates 8 polynomial streams per element (degrees 1..8 in
u = tanh(x)) and contracts them against per-dim weights on the PE.
The stream DAG is one-way ACT -> DVE so the scalar engine never waits
on the vector engine:
  ACT chain :  u = tanh(x),  q = u^2 (Square),  s4 = (2q-1)^2 = T2(u)^2
               (Square with scale=2, bias=-1)
  DVE muls  :  uq = u*q, us4 = u*s4, qs4 = q*s4, uqs4 = uq*s4, s44 = s4*s4
All streams live in [-1, 1] (fp16-friendly).  The d-contraction runs on
the PE as per-stream matvecs with 4 batch sub-blocks concurrent via
column tiling (PSUM rows 0/32/64/96 of one bank).
Host: transposes x to [D, B] fp16, folds hweights into coeffs, solves the
stream->Chebyshev transform with fp16-rounding compensation, and adds the
T0 constant plus output-row gather on the way out.
"""
import sys
import numpy as np

sys.path.insert(0, "/opt/trn_rl_repo")

import orjson
from contextlib import ExitStack

import concourse.bass as bass
from concourse import mybir
from concourse.tile import TileContext
from concourse.bass_utils import run_bass_kernel_spmd

F32 = mybir.dt.float32
F16 = mybir.dt.float16
AF = mybir.ActivationFunctionType
OP = mybir.AluOpType

B, D, DEG1 = 32768, 256, 9
NCORES = 8
BC = B // NCORES          # 4096 batch per core
BLK = 2048                # batch cols per super-block
NSB = BC // BLK           # super-blocks per core
NGRP = 4                  # PE column groups (batch sub-blocks in flight)
SUB = BLK // NGRP         # 512 cols per sub-block == one PSUM bank row

STREAMS = ["u", "q", "uq", "s4", "us4", "qs4", "uqs4", "s44"]

# ---- walrus workaround: split >1 sem-waits onto Drain carriers -------------
_MAXW = 1

def _split_waits(bir_json: bytes) -> bytes:
    d = orjson.loads(bir_json)
    for fn in d.get("functions", []):
        for bb in fn.get("blocks", []):
            out = []
            for ins in bb.get("instructions", []):
                si = ins.get("sync_info") or {}
                waits = si.get("on_wait") or []
                if len(waits) > _MAXW:
                    extra, keep = waits[:-_MAXW], waits[-_MAXW:]
                    for i in range(0, len(extra), _MAXW):
                        out.append({
                            "debug": ins.get("debug", 0),
                            "engine": ins["engine"], "ins": [], "outs": [],
                            "name": f"{ins['name']}_ws{i}", "opcode": "Drain",
                            "sync_info": {"on_update": [],
                                          "on_wait": extra[i:i + _MAXW]},
                        })
                    si["on_wait"] = keep
                out.append(ins)
            bb["instructions"] = out
    return orjson.dumps(d)

def _install_patch():
    import concourse.bass_utils as bu
    if getattr(bu, "_ws_patched", False):
        return
    orig = bu.compile_bir_kernel
    def patched(bir_json, tmpdir, neff_name="file.neff"):
        return orig(_split_waits(bir_json), tmpdir, neff_name)
    bu.compile_bir_kernel = patched
    bu._ws_patched = True
    try:
        import concourse.bass2jax as b2j
        if getattr(b2j, "compile_bir_kernel", None) is orig:
            b2j.compile_bir_kernel = patched
    except Exception:
        pass

# ---- basis transform (host) ------------------------------------------------
def _stream_polys():
    """Power-basis coefficients (in u) of each stream, index by degree 1..8."""
    P = np.polynomial.polynomial
    u = [0.0, 1.0]
    q = P.polymul(u, u)
    uq = P.polymul(u, q)
    t2 = P.polyadd(P.polymul([2.0], q), [-1.0])     # 2u^2 - 1
    s4 = P.polymul(t2, t2)
    us4 = P.polymul(u, s4)
    qs4 = P.polymul(q, s4)
    uqs4 = P.polymul(uq, s4)
    s44 = P.polymul(s4, s4)
    return {1: u, 2: q, 3: uq, 4: s4, 5: us4, 6: qs4, 7: uqs4, 8: s44}

def _basis_matrix():
    """A[:, t] = Chebyshev T_0..T_9 coefficients of the degree-t stream."""
    from numpy.polynomial import chebyshev as C
    A = np.zeros((9, 9))
    A[0, 0] = 1.0
    for t, poly in _stream_polys().items():
        c = C.poly2cheb(poly)
        A[: len(c), t] = c
    return A

# ---- device kernel ---------------------------------------------------------
def _build():
    nc = bass.Bass(enable_partition_id=False)
    xt = nc.declare_dram_parameter("xt", [D, BC], F16, isOutput=False)
    wv = nc.declare_dram_parameter("wv", [128, 16], F16, isOutput=False)
    y = nc.declare_dram_parameter("y", [NGRP, NSB * SUB], F32, isOutput=True)

    with TileContext(nc) as tc, ExitStack() as ctx:
        fp = ctx.enter_context(tc.tile_pool(name="feat", bufs=4))
        pp = ctx.enter_context(tc.tile_pool(name="ps", bufs=2, space="PSUM"))

        # all input DMAs issued up front, in use order; the first transfer is
        # a small [128, 512] so its completion semaphore fires early
        xfs = {}
        Q1 = 512
        order = [(c, sb) for sb in range(NSB) for c in range(2)]
        for c, sb in order:
            bs = sb * BLK
            xf = fp.tile([128, BLK], F16, tag="x", name=f"xf{c}_{sb}")
            eng = nc.sync
            if sb == 0 and c == 0:
                eng.dma_start(out=xf[:, 0:Q1], in_=xt[0:128, bs:bs + Q1])
                eng.dma_start(out=xf[:, Q1:BLK], in_=xt[0:128, bs + Q1:bs + BLK])
            else:
                eng.dma_start(out=xf[:],
                              in_=xt[128 * c:128 * (c + 1), bs:bs + BLK])
            xfs[(c, sb)] = xf

        wb = fp.tile([128, 16], F16, tag="wb", bufs=1)
        nc.gpsimd.dma_start(out=wb[:], in_=wv[:])
        bM1 = fp.tile([128, 1], F32, tag="bM1", bufs=1)
        nc.vector.memset(bM1[:], -1.0)

        res = fp.tile([128, NSB * SUB], F32, tag="res", bufs=1)
        tiles = {}
        for c, sb in order:
            xf = xfs[(c, sb)]
            st = {}
            u = fp.tile([128, BLK], F16, tag="u")
            q = fp.tile([128, BLK], F16, tag="q")
            s4 = fp.tile([128, BLK], F16, tag="s4")
            # pure ACT chain: tanh -> Square -> Square(2q-1)
            if sb == 0 and c == 0:
                nc.scalar.activation(u[:, 0:Q1], xf[:, 0:Q1], AF.Tanh)
                nc.scalar.activation(u[:, Q1:BLK], xf[:, Q1:BLK], AF.Tanh)
                nc.scalar.activation(q[:, 0:Q1], u[:, 0:Q1], AF.Square)
                nc.scalar.activation(q[:, Q1:BLK], u[:, Q1:BLK], AF.Square)
            else:
                nc.scalar.activation(u[:], xf[:], AF.Tanh)
                nc.scalar.activation(q[:], u[:], AF.Square)
            nc.scalar.activation(s4[:], q[:], AF.Square, bias=bM1[:], scale=2.0)
            # DVE products
            uq = fp.tile([128, BLK], F16, tag="uq")
            nc.vector.tensor_mul(uq[:], u[:], q[:])
            us4 = fp.tile([128, BLK], F16, tag="us4")
            nc.vector.tensor_mul(us4[:], u[:], s4[:])
            qs4 = fp.tile([128, BLK], F16, tag="qs4")
            nc.vector.tensor_mul(qs4[:], q[:], s4[:])
            uqs4 = fp.tile([128, BLK], F16, tag="uqs4")
            nc.vector.tensor_mul(uqs4[:], uq[:], s4[:])
            s44 = fp.tile([128, BLK], F16, tag="s44")
            if c == 1 and sb == NSB - 1:
                # last stream produced: split so the final MM rounds flush
                # incrementally instead of waiting on the full tile
                nc.vector.tensor_mul(s44[:, 0:BLK // 2], s4[:, 0:BLK // 2],
                                     s4[:, 0:BLK // 2])
                nc.vector.tensor_mul(s44[:, BLK // 2:], s4[:, BLK // 2:],
                                     s4[:, BLK // 2:])
            else:
                nc.vector.tensor_mul(s44[:], s4[:], s4[:])
            st = dict(u=u, q=q, uq=uq, s4=s4, us4=us4, qs4=qs4,
                      uqs4=uqs4, s44=s44)
            tiles[(c, sb)] = st

        psts = []
        for sb in range(NSB):
            ps = pp.tile([128, SUB], F32)
            psts.append(ps)
            nround = 2 * len(STREAMS)
            r = 0
            for sidx in range(len(STREAMS)):
                for c in range(2):
                    stream = tiles[(c, sb)][STREAMS[sidx]]
                    for g in range(NGRP):
                        nc.tensor.matmul(
                            ps[32 * g:32 * g + 1, :],
                            wb[:, c * 8 + sidx:c * 8 + sidx + 1],
                            stream[:, g * SUB:(g + 1) * SUB],
                            start=(r == 0), stop=(r == nround - 1),
                            skip_group_check=True,
                            tile_position=(0, 32 * g))
                    r += 1

        for sb in range(NSB):
            dst = res[:, sb * SUB:(sb + 1) * SUB]
            nc.scalar.activation(dst, psts[sb][:], AF.Identity)
        nc.sync.dma_start(out=y[:], in_=res[0:128:32, :])
    return nc

# ---- public entry ----------------------------------------------------------
def kernel(x, coeffs, hweights, _trace=False):
    _install_patch()
    x = np.asarray(x, dtype=np.float32)
    w = (coeffs.astype(np.float64).T @ hweights.astype(np.float64))  # [2304]
    W = w.reshape(D, DEG1)                                           # [d, k]
    # quantization-compensated solve: peel leading Chebyshev components in
    # decreasing degree; each stream's fp16 weight rounding is re-absorbed by
    # the lower-degree streams, leftover T0 becomes the host-side constant.
    A = _basis_matrix()
    Wc = W.astype(np.float64).copy()
    lam = np.zeros((D, DEG1))
    for t in range(DEG1 - 1, 0, -1):
        lt = Wc[:, t] / A[t, t]
        ltq = lt.astype(np.float16).astype(np.float64)
        Wc -= ltq[:, None] * A[:, t][None, :]
        lam[:, t] = ltq
    c0 = float(Wc[:, 0].sum())
    wv = np.zeros((128, 16), dtype=np.float16)
    for c in range(2):
        for sidx in range(8):
            wv[:, c * 8 + sidx] = lam[c * 128:(c + 1) * 128, sidx + 1]

    nc = _build()
    xT = np.ascontiguousarray(x.T.astype(np.float16))                # [D, B]
    in_maps = [{"xt": np.ascontiguousarray(xT[:, i * BC:(i + 1) * BC]),
                "wv": wv} for i in range(NCORES)]
    res = run_bass_kernel_spmd(nc, in_maps, core_ids=list(range(NCORES)),
                               trace=_trace)
    # y[g, sb*SUB + i] holds batch col sb*BLK + g*SUB + i of this core;
    # the T0 constant c0 is added here (it is global, so host-side is free)
    parts = []
    for i in range(NCORES):
        yc = res.results[i]["y"].astype(np.float64) + c0   # [NGRP, NSB*SUB]
        parts.append(yc.reshape(NGRP, NSB, SUB).transpose(1, 0, 2).reshape(BC))
    out = np.concatenate(parts)
    if _trace:
        kernel._last = res
    return out.astype(np.float32)
